# revision 42
# baseline (speedup 1.0000x reference)
"""MoE MLP (E=4, top-2 routing) Trainium2 kernel, 8 NeuronCores.

Strategy (expert-parallel x tensor-parallel): core c handles expert
e = c // 2 and FFN half = c % 2.  Each core receives ALL tokens routed to
its expert (routing_map column e), computes

    z = p_e * gelu(x @ w1[e][:, half]) @ w2[e][half, :]

for its half of the FFN dimension, and the host scatter-adds the four
partials per token (2 experts x 2 halves) plus the residual.  This is the
standard all-to-all dispatch / combine of expert parallelism, with the
dispatch/combine permutation done host-side (full-I/O contract).

Matmuls run in fp8 (e4m3) with DoubleRow perf mode: each instruction
contracts 256 elements (2 k-tiles of 128) at 0.5 cycles per output row
(4x the bf16 rate).  To stay inside the 2e-2 error budget, three of the
four quantization error sources are compensated with extra DoubleRow
passes on the residuals (hi/lo splitting), all accumulated in the same
PSUM chain:

    fc1 chain: x_hi@w1_hi + x_lo@w1_hi + x_hi@w1_lo      (8+K1 DR steps)
    fc2 chain: a@w2_hi + a@w2_lo                         (8+K2 DR steps)

which leaves the gelu-output (a) quantization as the dominant error
source (~1.6e-2 on the max-abs metric at K1=4, K2=2).  Weights are pre-scaled (w1*32, w2*64)
so e4m3 stays in its normal range; the scales are folded into the gelu
activation scale and the per-token prob factors.  fc2 chains are
interleaved between fc1 token-chunk groups so the PE never waits for the
activation engine to drain gelu work.
"""
import sys

import numpy as np

try:
    import concourse.bass as bass  # noqa: F401
except Exception:
    sys.path.insert(0, "/opt/trn_rl_repo")

import ml_dtypes

import concourse.bacc as bacc
import concourse.bass as bass
import concourse.mybir as mybir
import concourse.tile as tile
from concourse.bass_utils import run_bass_kernel_spmd

S, B, H, F, E = 1024, 2, 1024, 4096, 4
T = S * B
N_CORES = 8
FH = F // 2          # per-core FFN slice
NH = H // 128        # 8 h-tiles
NF1 = FH // 128      # 16 f-tiles per core (fc1 outputs)
NQ = NF1 // 2        # 8 DoubleRow contraction steps for fc2
K1 = 4               # w1_lo correction DR steps (0..4)
K2 = 2               # w2_lo correction DR steps (0..8)
WARM = 70            # PE warmup DoubleRow steps
FIRSTC = 512         # first token-chunk width
S1 = 32.0            # w1 pre-scale (w1 ~ N(0, 1/32))
S2 = 64.0            # w2 pre-scale (w2 ~ N(0, 1/64))
F8 = mybir.dt.float8e4
F8_NP = ml_dtypes.float8_e4m3

_NC_CACHE = {}


def _build_nc(C):
    key = (C,)
    if key in _NC_CACHE:
        return _NC_CACHE[key]
    NT = -(-C // 128)
    f32 = mybir.dt.float32
    bf16 = mybir.dt.bfloat16
    DR = mybir.MatmulPerfMode.DoubleRow
    # token chunks for fc1 (psum bank = 512 fp32); a small first chunk
    # lets compute start earlier behind the initial DMAs
    bounds = [0, min(FIRSTC, C)]
    while bounds[-1] < C:
        bounds.append(min(bounds[-1] + 512, C))
    chunks = list(zip(bounds, bounds[1:]))

    nc = bacc.Bacc("TRN2", target_bir_lowering=False, debug=False,
                   num_devices=N_CORES)
    xh_d = nc.declare_dram_parameter("xh", [128, NH, C], F8, isOutput=False)
    xl_d = nc.declare_dram_parameter("xl", [128, NH, C], F8, isOutput=False)
    w1h_d = nc.declare_dram_parameter("w1h", [128, NF1 * 4 * 2, 128], F8,
                                      isOutput=False)
    w1l_d = nc.declare_dram_parameter("w1l", [128, NF1 * 4 * 2, 128], F8,
                                      isOutput=False)
    w2h_d = nc.declare_dram_parameter("w2h", [128, NQ * 2, H], F8,
                                      isOutput=False)
    w2l_d = nc.declare_dram_parameter("w2l", [128, NQ * 2, H], F8,
                                      isOutput=False)
    pp_d = nc.declare_dram_parameter("pp", [128, C], f32, isOutput=False)
    out_d = nc.declare_dram_parameter("out", [128, NH, C], bf16,
                                      isOutput=True)

    with tile.TileContext(nc) as tc:
        with (
            tc.tile_pool(name="resident", bufs=1) as rpool,
            tc.tile_pool(name="pa", bufs=4, space="PSUM") as papool,
            tc.tile_pool(name="py", bufs=4, space="PSUM") as pypool,
        ):
            # PE warmup: keep the tensor engine continuously busy from t~0
            # so its p-state ramp (slow first 3us) completes during the
            # initial DMA window instead of during real work.
            wm_w = rpool.tile([128, 2, 128], F8, tag="wmw")
            nc.vector.memset(wm_w[:], 0.0)
            wm_p = papool.tile([128, 512], mybir.dt.float32, tag="pa")
            for i in range(WARM):
                nc.tensor.matmul(wm_p[:, 0:128], wm_w[:], wm_w[:],
                                 start=(i == 0), stop=(i == WARM - 1),
                                 perf_mode=DR)

            xh_sb = rpool.tile([128, NH, C], F8, tag="xh")
            xl_sb = rpool.tile([128, NH, C], F8, tag="xl")
            w1h_sb = rpool.tile([128, NF1 * 4 * 2, 128], F8, tag="w1h")
            w1l_sb = rpool.tile([128, NF1 * 4 * 2, 128], F8, tag="w1l")
            w2h_sb = rpool.tile([128, NQ * 2, H], F8, tag="w2h")
            w2l_sb = rpool.tile([128, NQ * 2, H], F8, tag="w2l")
            pp_sb = rpool.tile([128, C], f32, tag="pp")
            a_sb = rpool.tile([128, NF1, C], F8, tag="a")
            z_sb = rpool.tile([128, NH, C], bf16, tag="z")

            # --- input DMAs, ordered so fc1 can start early ---
            c1_0 = chunks[0][1]
            nc.sync.dma_start(xh_sb[:, :, 0:c1_0], xh_d[:, :, 0:c1_0])
            nc.sync.dma_start(w1h_sb[:, 0:8, :], w1h_d[:, 0:8, :])
            nc.sync.dma_start(xl_sb[:, :, 0:c1_0], xl_d[:, :, 0:c1_0])
            if K1:
                nc.sync.dma_start(w1l_sb[:, 0:8, :], w1l_d[:, 0:8, :])
            for ft in range(1, 4):  # rest of first w1 quarter, per tile
                nc.sync.dma_start(w1h_sb[:, ft * 8:(ft + 1) * 8, :],
                                  w1h_d[:, ft * 8:(ft + 1) * 8, :])
                if K1:
                    nc.sync.dma_start(w1l_sb[:, ft * 8:(ft + 1) * 8, :],
                                      w1l_d[:, ft * 8:(ft + 1) * 8, :])
            for q4 in range(1, 4):
                nc.sync.dma_start(w1h_sb[:, q4 * 32:(q4 + 1) * 32, :],
                                  w1h_d[:, q4 * 32:(q4 + 1) * 32, :])
                if K1:
                    nc.sync.dma_start(w1l_sb[:, q4 * 32:(q4 + 1) * 32, :],
                                      w1l_d[:, q4 * 32:(q4 + 1) * 32, :])
            for c0, c1 in chunks[1:]:
                nc.sync.dma_start(xh_sb[:, :, c0:c1], xh_d[:, :, c0:c1])
                nc.sync.dma_start(xl_sb[:, :, c0:c1], xl_d[:, :, c0:c1])
            nc.sync.dma_start(w2h_sb[:, 0:NQ, :], w2h_d[:, 0:NQ, :])
            nc.sync.dma_start(w2h_sb[:, NQ:2 * NQ, :], w2h_d[:, NQ:2 * NQ, :])
            if K2:
                nc.sync.dma_start(w2l_sb[:, 0:NQ, :], w2l_d[:, 0:NQ, :])
                nc.sync.dma_start(w2l_sb[:, NQ:2 * NQ, :],
                                  w2l_d[:, NQ:2 * NQ, :])
            nc.sync.dma_start(pp_sb[:], pp_d[:, :])

            def fc1_group(g):
                c0, c1 = chunks[g]
                for ft in range(NF1):
                    pa = papool.tile([128, c1 - c0], f32, tag="pa")
                    nsteps = 8 + K1
                    step = 0
                    for j in range(4):  # x_hi @ w1_hi
                        nc.tensor.matmul(
                            pa[:, :],
                            w1h_sb[:, (ft * 4 + j) * 2:(ft * 4 + j) * 2 + 2, :],
                            xh_sb[:, 2 * j:2 * j + 2, c0:c1],
                            start=(step == 0), stop=(step == nsteps - 1),
                            perf_mode=DR)
                        step += 1
                    for j in range(4):  # x_lo @ w1_hi
                        nc.tensor.matmul(
                            pa[:, :],
                            w1h_sb[:, (ft * 4 + j) * 2:(ft * 4 + j) * 2 + 2, :],
                            xl_sb[:, 2 * j:2 * j + 2, c0:c1],
                            start=(step == 0), stop=(step == nsteps - 1),
                            perf_mode=DR)
                        step += 1
                    for j in range(K1):  # x_hi @ w1_lo
                        nc.tensor.matmul(
                            pa[:, :],
                            w1l_sb[:, (ft * 4 + j) * 2:(ft * 4 + j) * 2 + 2, :],
                            xh_sb[:, 2 * j:2 * j + 2, c0:c1],
                            start=(step == 0), stop=(step == nsteps - 1),
                            perf_mode=DR)
                        step += 1
                    nc.scalar.activation(
                        a_sb[:, ft, c0:c1], pa[:, :],
                        mybir.ActivationFunctionType.Gelu, scale=1.0 / S1)

            def fc2_group(g):
                c0, c1 = chunks[g]
                for ht in range(NH):
                    py = pypool.tile([128, c1 - c0], f32, tag="py")
                    nsteps = NQ + K2
                    step = 0
                    for q in range(NQ):  # w2_hi.T @ a
                        nc.tensor.matmul(
                            py[:, :],
                            w2h_sb[:, 2 * q:2 * q + 2,
                                   ht * 128:(ht + 1) * 128],
                            a_sb[:, 2 * q:2 * q + 2, c0:c1],
                            start=(step == 0), stop=(step == nsteps - 1),
                            perf_mode=DR)
                        step += 1
                    for q in range(K2):  # w2_lo.T @ a
                        nc.tensor.matmul(
                            py[:, :],
                            w2l_sb[:, 2 * q:2 * q + 2,
                                   ht * 128:(ht + 1) * 128],
                            a_sb[:, 2 * q:2 * q + 2, c0:c1],
                            start=(step == 0), stop=(step == nsteps - 1),
                            perf_mode=DR)
                        step += 1
                    nc.vector.tensor_tensor(
                        z_sb[:, ht, c0:c1], py[:, :], pp_sb[:, c0:c1],
                        mybir.AluOpType.mult)
                    # z out split 6+2: the small last piece shortens the
                    # critical path after the final evict; narrow groups go
                    # in one DMA (HWDGE issue cost dominates)
                    if c1 - c0 > 128 and ht == 5:
                        nc.sync.dma_start(out_d[:, 0:6, c0:c1],
                                          z_sb[:, 0:6, c0:c1])
                    elif ht == NH - 1:
                        h0 = 6 if c1 - c0 > 128 else 0
                        nc.sync.dma_start(out_d[:, h0:NH, c0:c1],
                                          z_sb[:, h0:NH, c0:c1])

            # interleave: fc2 of chunk group g runs while fc1 of group g+1
            # feeds the activation engine, keeping the PE busy throughout
            ngroups = len(chunks)
            fc1_group(0)
            for g in range(1, ngroups):
                fc1_group(g)
                fc2_group(g - 1)
            fc2_group(ngroups - 1)
    nc.compile()
    _NC_CACHE[key] = nc
    return nc


def _q8(v):
    return np.asarray(v).astype(F8_NP)


def kernel(hidden_states, mlp_residual, probs, routing_map, w1, w2,
           _trace=False):
    hidden_states = np.ascontiguousarray(np.asarray(hidden_states, np.float32))
    mlp_residual = np.asarray(mlp_residual, np.float32)
    probs = np.asarray(probs, np.float32)
    routing_map = np.asarray(routing_map, bool)
    w1 = np.asarray(w1, np.float32)
    w2 = np.asarray(w2, np.float32)

    x = hidden_states.reshape(T, H)
    idx = [np.nonzero(routing_map[:, e])[0] for e in range(E)]
    C = max(128, max(len(i) for i in idx))

    in_maps = []
    for c in range(N_CORES):
        e, half = c // 2, c % 2
        ids, n = idx[e], len(idx[e])
        # x^T blocked: xt[p, ht, col] = x[ids[col], ht*128 + p]
        xe = x[ids].T.reshape(NH, 128, n).transpose(1, 0, 2)
        xh = np.zeros((128, NH, C), F8_NP)
        xl = np.zeros((128, NH, C), F8_NP)
        xh[:, :, :n] = _q8(xe)
        xl[:, :, :n] = _q8(xe - xh[:, :, :n].astype(np.float32))
        # w1 half, DoubleRow blocked: [p, (ft, j, i), f],  hh = (2j+i)*128+p
        w1s = w1[e][:, half * FH:(half + 1) * FH] * S1
        w1hq = _q8(w1s)
        w1lq = _q8(w1s - w1hq.astype(np.float32))

        def blk1(a):
            return np.ascontiguousarray(
                a.reshape(4, 2, 128, NF1, 128).transpose(2, 3, 0, 1, 4)
                .reshape(128, NF1 * 4 * 2, 128))
        # w2 half, DoubleRow blocked: [p, (q, i), h],  ff = (2q+i)*128+p
        w2s = w2[e][half * FH:(half + 1) * FH, :] * S2
        w2hq = _q8(w2s)
        w2lq = _q8(w2s - w2hq.astype(np.float32))

        def blk2(a):
            return np.ascontiguousarray(
                a.reshape(NQ, 2, 128, H).transpose(2, 0, 1, 3)
                .reshape(128, NQ * 2, H))
        pcol = np.zeros(C, np.float32)
        pcol[:n] = probs[ids, e] / S2
        pp = np.ascontiguousarray(np.broadcast_to(pcol, (128, C)))
        in_maps.append({"xh": xh, "xl": xl, "w1h": blk1(w1hq),
                        "w1l": blk1(w1lq), "w2h": blk2(w2hq),
                        "w2l": blk2(w2lq), "pp": pp})

    nc = _build_nc(C)
    r = run_bass_kernel_spmd(nc, in_maps, list(range(N_CORES)),
                             trace=_trace)

    out = mlp_residual.reshape(T, H).copy()
    for e in range(E):
        ids, n = idx[e], len(idx[e])
        if n == 0:
            continue
        # z layout: [p, ht, c] with h = ht*128 + p
        z = (np.asarray(r.results[2 * e]["out"], np.float32)
             + np.asarray(r.results[2 * e + 1]["out"], np.float32))
        out[ids] += z.transpose(2, 1, 0).reshape(C, H)[:n]
    result = out.reshape(S, B, H)
    if _trace:
        return result, r
    return result


# revision 43
# speedup vs baseline: 1.0252x; 1.0252x over previous
"""MoE MLP (E=4, top-2 routing) Trainium2 kernel, 8 NeuronCores.

Strategy (expert-parallel x tensor-parallel): core c handles expert
e = c // 2 and FFN half = c % 2.  Each core receives ALL tokens routed to
its expert (routing_map column e), computes

    z = p_e * gelu(x @ w1[e][:, half]) @ w2[e][half, :]

for its half of the FFN dimension, and the host scatter-adds the four
partials per token (2 experts x 2 halves) plus the residual.  This is the
standard all-to-all dispatch / combine of expert parallelism, with the
dispatch/combine permutation done host-side (full-I/O contract).

Matmuls run in fp8 (e4m3) with DoubleRow perf mode: each instruction
contracts 256 elements (2 k-tiles of 128) at 0.5 cycles per output row
(4x the bf16 rate).  To stay inside the 2e-2 error budget, three of the
four quantization error sources are compensated with extra DoubleRow
passes on the residuals (hi/lo splitting), all accumulated in the same
PSUM chain:

    fc1 chain: x_hi@w1_hi + x_lo@w1_hi + x_hi@w1_lo      (8+K1 DR steps)
    fc2 chain: a@w2_hi + a@w2_lo                         (8+K2 DR steps)

which leaves the gelu-output (a) quantization as the dominant error
source (~1.6e-2 on the max-abs metric at K1=4, K2=2).  Weights are pre-scaled (w1*32, w2*64)
so e4m3 stays in its normal range; the scales are folded into the gelu
activation scale and the per-token prob factors.  fc2 chains are
interleaved between fc1 token-chunk groups so the PE never waits for the
activation engine to drain gelu work.
"""
import sys

import numpy as np

try:
    import concourse.bass as bass  # noqa: F401
except Exception:
    sys.path.insert(0, "/opt/trn_rl_repo")

import ml_dtypes

import concourse.bacc as bacc
import concourse.bass as bass
import concourse.mybir as mybir
import concourse.tile as tile
from concourse.bass_utils import run_bass_kernel_spmd

S, B, H, F, E = 1024, 2, 1024, 4096, 4
T = S * B
N_CORES = 8
FH = F // 2          # per-core FFN slice
NH = H // 128        # 8 h-tiles
NF1 = FH // 128      # 16 f-tiles per core (fc1 outputs)
NQ = NF1 // 2        # 8 DoubleRow contraction steps for fc2
K1 = 4               # w1_lo correction DR steps (0..4)
K2 = 1               # w2_lo correction DR steps (0..8)
WARM = 70            # PE warmup DoubleRow steps
FIRSTC = 512         # first token-chunk width
S1 = 32.0            # w1 pre-scale (w1 ~ N(0, 1/32))
S2 = 64.0            # w2 pre-scale (w2 ~ N(0, 1/64))
F8 = mybir.dt.float8e4
F8_NP = ml_dtypes.float8_e4m3

_NC_CACHE = {}


def _build_nc(C):
    key = (C,)
    if key in _NC_CACHE:
        return _NC_CACHE[key]
    NT = -(-C // 128)
    f32 = mybir.dt.float32
    bf16 = mybir.dt.bfloat16
    DR = mybir.MatmulPerfMode.DoubleRow
    # token chunks for fc1 (psum bank = 512 fp32); a small first chunk
    # lets compute start earlier behind the initial DMAs
    bounds = [0, min(FIRSTC, C)]
    while bounds[-1] < C:
        bounds.append(min(bounds[-1] + 512, C))
    chunks = list(zip(bounds, bounds[1:]))

    nc = bacc.Bacc("TRN2", target_bir_lowering=False, debug=False,
                   num_devices=N_CORES)
    xh_d = nc.declare_dram_parameter("xh", [128, NH, C], F8, isOutput=False)
    xl_d = nc.declare_dram_parameter("xl", [128, NH, C], F8, isOutput=False)
    w1h_d = nc.declare_dram_parameter("w1h", [128, NF1 * 4 * 2, 128], F8,
                                      isOutput=False)
    w1l_d = nc.declare_dram_parameter("w1l", [128, NF1 * 4 * 2, 128], F8,
                                      isOutput=False)
    w2h_d = nc.declare_dram_parameter("w2h", [128, NQ * 2, H], F8,
                                      isOutput=False)
    w2l_d = nc.declare_dram_parameter("w2l", [128, NQ * 2, H], F8,
                                      isOutput=False)
    pp_d = nc.declare_dram_parameter("pp", [128, C], f32, isOutput=False)
    out_d = nc.declare_dram_parameter("out", [128, NH, C], bf16,
                                      isOutput=True)

    with tile.TileContext(nc) as tc:
        with (
            tc.tile_pool(name="resident", bufs=1) as rpool,
            tc.tile_pool(name="pa", bufs=4, space="PSUM") as papool,
            tc.tile_pool(name="py", bufs=4, space="PSUM") as pypool,
        ):
            # PE warmup: keep the tensor engine continuously busy from t~0
            # so its p-state ramp (slow first 3us) completes during the
            # initial DMA window instead of during real work.
            wm_w = rpool.tile([128, 2, 128], F8, tag="wmw")
            nc.vector.memset(wm_w[:], 0.0)
            wm_p = papool.tile([128, 512], mybir.dt.float32, tag="pa")
            for i in range(WARM):
                nc.tensor.matmul(wm_p[:, 0:128], wm_w[:], wm_w[:],
                                 start=(i == 0), stop=(i == WARM - 1),
                                 perf_mode=DR)

            xh_sb = rpool.tile([128, NH, C], F8, tag="xh")
            xl_sb = rpool.tile([128, NH, C], F8, tag="xl")
            w1h_sb = rpool.tile([128, NF1 * 4 * 2, 128], F8, tag="w1h")
            w1l_sb = rpool.tile([128, NF1 * 4 * 2, 128], F8, tag="w1l")
            w2h_sb = rpool.tile([128, NQ * 2, H], F8, tag="w2h")
            w2l_sb = rpool.tile([128, NQ * 2, H], F8, tag="w2l")
            pp_sb = rpool.tile([128, C], f32, tag="pp")
            a_sb = rpool.tile([128, NF1, C], F8, tag="a")
            z_sb = rpool.tile([128, NH, C], bf16, tag="z")

            # --- input DMAs, ordered so fc1 can start early ---
            c1_0 = chunks[0][1]
            nc.sync.dma_start(xh_sb[:, :, 0:c1_0], xh_d[:, :, 0:c1_0])
            nc.sync.dma_start(w1h_sb[:, 0:8, :], w1h_d[:, 0:8, :])
            nc.sync.dma_start(xl_sb[:, :, 0:c1_0], xl_d[:, :, 0:c1_0])
            if K1:
                nc.sync.dma_start(w1l_sb[:, 0:8, :], w1l_d[:, 0:8, :])
            for ft in range(1, 4):  # rest of first w1 quarter, per tile
                nc.sync.dma_start(w1h_sb[:, ft * 8:(ft + 1) * 8, :],
                                  w1h_d[:, ft * 8:(ft + 1) * 8, :])
                if K1:
                    nc.sync.dma_start(w1l_sb[:, ft * 8:(ft + 1) * 8, :],
                                      w1l_d[:, ft * 8:(ft + 1) * 8, :])
            for q4 in range(1, 4):
                nc.sync.dma_start(w1h_sb[:, q4 * 32:(q4 + 1) * 32, :],
                                  w1h_d[:, q4 * 32:(q4 + 1) * 32, :])
                if K1:
                    nc.sync.dma_start(w1l_sb[:, q4 * 32:(q4 + 1) * 32, :],
                                      w1l_d[:, q4 * 32:(q4 + 1) * 32, :])
            for c0, c1 in chunks[1:]:
                nc.sync.dma_start(xh_sb[:, :, c0:c1], xh_d[:, :, c0:c1])
                nc.sync.dma_start(xl_sb[:, :, c0:c1], xl_d[:, :, c0:c1])
            nc.sync.dma_start(w2h_sb[:, 0:NQ, :], w2h_d[:, 0:NQ, :])
            nc.sync.dma_start(w2h_sb[:, NQ:2 * NQ, :], w2h_d[:, NQ:2 * NQ, :])
            if K2:
                nc.sync.dma_start(w2l_sb[:, 0:NQ, :], w2l_d[:, 0:NQ, :])
                nc.sync.dma_start(w2l_sb[:, NQ:2 * NQ, :],
                                  w2l_d[:, NQ:2 * NQ, :])
            nc.sync.dma_start(pp_sb[:], pp_d[:, :])

            def fc1_group(g):
                c0, c1 = chunks[g]
                for ft in range(NF1):
                    pa = papool.tile([128, c1 - c0], f32, tag="pa")
                    nsteps = 8 + K1
                    step = 0
                    for j in range(4):  # x_hi @ w1_hi
                        nc.tensor.matmul(
                            pa[:, :],
                            w1h_sb[:, (ft * 4 + j) * 2:(ft * 4 + j) * 2 + 2, :],
                            xh_sb[:, 2 * j:2 * j + 2, c0:c1],
                            start=(step == 0), stop=(step == nsteps - 1),
                            perf_mode=DR)
                        step += 1
                    for j in range(4):  # x_lo @ w1_hi
                        nc.tensor.matmul(
                            pa[:, :],
                            w1h_sb[:, (ft * 4 + j) * 2:(ft * 4 + j) * 2 + 2, :],
                            xl_sb[:, 2 * j:2 * j + 2, c0:c1],
                            start=(step == 0), stop=(step == nsteps - 1),
                            perf_mode=DR)
                        step += 1
                    for j in range(K1):  # x_hi @ w1_lo
                        nc.tensor.matmul(
                            pa[:, :],
                            w1l_sb[:, (ft * 4 + j) * 2:(ft * 4 + j) * 2 + 2, :],
                            xh_sb[:, 2 * j:2 * j + 2, c0:c1],
                            start=(step == 0), stop=(step == nsteps - 1),
                            perf_mode=DR)
                        step += 1
                    nc.scalar.activation(
                        a_sb[:, ft, c0:c1], pa[:, :],
                        mybir.ActivationFunctionType.Gelu, scale=1.0 / S1)

            def fc2_group(g):
                c0, c1 = chunks[g]
                for ht in range(NH):
                    py = pypool.tile([128, c1 - c0], f32, tag="py")
                    nsteps = NQ + K2
                    step = 0
                    for q in range(NQ):  # w2_hi.T @ a
                        nc.tensor.matmul(
                            py[:, :],
                            w2h_sb[:, 2 * q:2 * q + 2,
                                   ht * 128:(ht + 1) * 128],
                            a_sb[:, 2 * q:2 * q + 2, c0:c1],
                            start=(step == 0), stop=(step == nsteps - 1),
                            perf_mode=DR)
                        step += 1
                    for q in range(K2):  # w2_lo.T @ a
                        nc.tensor.matmul(
                            py[:, :],
                            w2l_sb[:, 2 * q:2 * q + 2,
                                   ht * 128:(ht + 1) * 128],
                            a_sb[:, 2 * q:2 * q + 2, c0:c1],
                            start=(step == 0), stop=(step == nsteps - 1),
                            perf_mode=DR)
                        step += 1
                    nc.vector.tensor_tensor(
                        z_sb[:, ht, c0:c1], py[:, :], pp_sb[:, c0:c1],
                        mybir.AluOpType.mult)
                    # z out split 6+2: the small last piece shortens the
                    # critical path after the final evict; narrow groups go
                    # in one DMA (HWDGE issue cost dominates)
                    if c1 - c0 > 128 and ht == 5:
                        nc.sync.dma_start(out_d[:, 0:6, c0:c1],
                                          z_sb[:, 0:6, c0:c1])
                    elif ht == NH - 1:
                        h0 = 6 if c1 - c0 > 128 else 0
                        nc.sync.dma_start(out_d[:, h0:NH, c0:c1],
                                          z_sb[:, h0:NH, c0:c1])

            # interleave: fc2 of chunk group g runs while fc1 of group g+1
            # feeds the activation engine, keeping the PE busy throughout
            ngroups = len(chunks)
            fc1_group(0)
            for g in range(1, ngroups):
                fc1_group(g)
                fc2_group(g - 1)
            fc2_group(ngroups - 1)
    nc.compile()
    _NC_CACHE[key] = nc
    return nc


def _q8(v):
    return np.asarray(v).astype(F8_NP)


def kernel(hidden_states, mlp_residual, probs, routing_map, w1, w2,
           _trace=False):
    hidden_states = np.ascontiguousarray(np.asarray(hidden_states, np.float32))
    mlp_residual = np.asarray(mlp_residual, np.float32)
    probs = np.asarray(probs, np.float32)
    routing_map = np.asarray(routing_map, bool)
    w1 = np.asarray(w1, np.float32)
    w2 = np.asarray(w2, np.float32)

    x = hidden_states.reshape(T, H)
    idx = [np.nonzero(routing_map[:, e])[0] for e in range(E)]
    C = max(128, max(len(i) for i in idx))

    in_maps = []
    for c in range(N_CORES):
        e, half = c // 2, c % 2
        ids, n = idx[e], len(idx[e])
        # x^T blocked: xt[p, ht, col] = x[ids[col], ht*128 + p]
        xe = x[ids].T.reshape(NH, 128, n).transpose(1, 0, 2)
        xh = np.zeros((128, NH, C), F8_NP)
        xl = np.zeros((128, NH, C), F8_NP)
        xh[:, :, :n] = _q8(xe)
        xl[:, :, :n] = _q8(xe - xh[:, :, :n].astype(np.float32))
        # w1 half, DoubleRow blocked: [p, (ft, j, i), f],  hh = (2j+i)*128+p
        w1s = w1[e][:, half * FH:(half + 1) * FH] * S1
        w1hq = _q8(w1s)
        w1lq = _q8(w1s - w1hq.astype(np.float32))

        def blk1(a):
            return np.ascontiguousarray(
                a.reshape(4, 2, 128, NF1, 128).transpose(2, 3, 0, 1, 4)
                .reshape(128, NF1 * 4 * 2, 128))
        # w2 half, DoubleRow blocked: [p, (q, i), h],  ff = (2q+i)*128+p
        w2s = w2[e][half * FH:(half + 1) * FH, :] * S2
        w2hq = _q8(w2s)
        w2lq = _q8(w2s - w2hq.astype(np.float32))

        def blk2(a):
            return np.ascontiguousarray(
                a.reshape(NQ, 2, 128, H).transpose(2, 0, 1, 3)
                .reshape(128, NQ * 2, H))
        pcol = np.zeros(C, np.float32)
        pcol[:n] = probs[ids, e] / S2
        pp = np.ascontiguousarray(np.broadcast_to(pcol, (128, C)))
        in_maps.append({"xh": xh, "xl": xl, "w1h": blk1(w1hq),
                        "w1l": blk1(w1lq), "w2h": blk2(w2hq),
                        "w2l": blk2(w2lq), "pp": pp})

    nc = _build_nc(C)
    r = run_bass_kernel_spmd(nc, in_maps, list(range(N_CORES)),
                             trace=_trace)

    out = mlp_residual.reshape(T, H).copy()
    for e in range(E):
        ids, n = idx[e], len(idx[e])
        if n == 0:
            continue
        # z layout: [p, ht, c] with h = ht*128 + p
        z = (np.asarray(r.results[2 * e]["out"], np.float32)
             + np.asarray(r.results[2 * e + 1]["out"], np.float32))
        out[ids] += z.transpose(2, 1, 0).reshape(C, H)[:n]
    result = out.reshape(S, B, H)
    if _trace:
        return result, r
    return result
